# revision 4
# baseline (speedup 1.0000x reference)
"""Trainium2 Bass kernel for nn_AutoMemoryModule (scatter_memory) — v2.

Two-launch pipeline over 8 NeuronCores:

Launch 1 (8 cores, SPMD): K-sharded first-layer matvec at the HBM roofline.
  w1 is split on the host into bf16 hi/lo planes (exact 4-term product,
  |err| ~ 2^-18 relative — same accuracy class as the HW fp32 path) and
  streamed as the bf16 moving operand: per MM, lhsT = 8 embedding
  component columns (2 K-chunks x (s0h, s1h, s0l, s1l)), rhs = 256 w1
  columns (2 K-chunks x [wh|wl]).  128 accumulating matmuls -> psum[8,256]
  per core (~14 us tensor, hidden under the ~24 us 8 MB DMA stream).
  emb + w1 blocks go on one HWDGE ring in order (emb + small first block
  first) so the PE starts ~7 us in; host folds the 8 psum tiles.

Launch 2 (1 core): second layer + scatter-max dedup + total-order rank,
  everything in COLUMN space ([128, 4] tiles: q = 128k + i) so DVE ops
  run on 128 lanes:
  - z column tile via 16 accumulating K=64 bf16 matmuls (w2 chunk
    stationary, h hi/lo streaming), b2 folded in by the psum->sbuf add.
  - duplicate-token group max: a handful of [1,1] pairwise maxes at
    host-known positions (token-only data).
  - val = gate ? groupmax : -BIG  (2 [128,4] ops)
  - rank_q = #{p: val_p > val_q}: 4 ident-matmul transposes -> val row,
    PE broadcast to [128,512], chunks 0-1 counted via DVE is_lt -> bf16
    matmul, chunks 2-3 via ACT sign() -> bf16 matmul (host decodes
    rank = rkA + (256 - eq - ... + rkB)/2).
  Host scatters (tok, sigmoid(val)) by rank.

Toolchain discipline:
  - one semaphore wait per instruction (_split_multi_waits hoists extras
    onto same-engine NOPs)
  - matmul start=True clears the whole PSUM BANK -> interleaved
    accumulation groups get separate banks; within a group only the
    first-emitted matmul carries start=True.
  - _hoist_preamble_dmas moves the leading wait-free DMA triggers into
    the framework preamble (right after the engine's register loads),
    overlapping the ~3 us engine barrier with the first transfers.
"""
import sys
import numpy as np

sys.path.insert(0, "/opt/trn_rl_repo")

import ml_dtypes
import concourse.bass as bass
import concourse.tile as tile
from concourse import mybir
from concourse.bass_utils import run_bass_kernel_spmd

F32 = mybir.dt.float32
BF16 = mybir.dt.bfloat16
NEG = np.float32(-1e20)
BIG = 1.0e20
VOCAB, MSL, EMB = 32000, 256, 1024
NCORES = 8
KTOT = EMB * MSL                 # 262144
KSH = KTOT // NCORES             # 32768 rows per core
CH = KSH // 128                  # 256 K-chunks per core
NMM = CH // 2                    # 128 matmuls (2 chunks each)
# w1 DMA blocks in chunks (sum = 256): small first blocks for fast PE start
BLOCKS = [8, 24, 32, 32, 32, 32, 32, 32, 32]
assert sum(BLOCKS) == CH

Alu = mybir.AluOpType
Act = mybir.ActivationFunctionType
BF = ml_dtypes.bfloat16


def _split_multi_waits(nc):
    """Walrus rejects instructions carrying more than one sem wait.  Hoist
    all but one wait of every such instruction onto same-engine NOPs."""
    import copy
    templates = {}
    for fn in nc.m.functions:
        for bb in fn.blocks:
            for ins in bb.instructions:
                if type(ins).__name__ == "InstEventSemaphore" \
                        and ins.engine not in templates:
                    templates[ins.engine] = ins
    n = [0]

    def make_nop(eng, w):
        tpl = templates[eng]
        nop = copy.deepcopy(tpl)
        n[0] += 1
        nop.name = f"WS-{n[0]}"
        nop.sync_info = mybir.SyncInfo(on_wait=[w], on_update=[])
        return nop

    for fn in nc.m.functions:
        for bb in fn.blocks:
            out = []
            for ins in bb.instructions:
                si = getattr(ins, "sync_info", None)
                if si is not None and si.on_wait and len(si.on_wait) > 1:
                    waits = list(si.on_wait)
                    for w in waits[:-1]:
                        out.append(make_nop(ins.engine, w))
                    si.on_wait = [waits[-1]]
                out.append(ins)
            bb.instructions[:] = out


def _hoist_preamble_dmas(nc, max_moved=4, after_first_rm=False):
    """Move each engine's leading wait-free DMA triggers from the main
    block into the framework preamble.

    after_first_rm=False: insert after the engine's barrier-participation
    EventSemaphore — the ~0.7 us/trigger dispatch does not delay the
    other engines (right choice when on-core compute is the critical
    path, i.e. the tail launch).

    after_first_rm=True: insert right after the engine's FIRST
    RegisterMove (the DRAM-base $R load) — transfers start ~2 us
    earlier, at the price of delaying the engine barrier by the trigger
    dispatch time (right choice when the launch is DMA-bound end-to-end,
    i.e. the stream launch)."""
    fn = nc.m.functions[0]
    blocks = fn.blocks
    pre, main = None, None
    for bb in blocks:
        names = [type(i).__name__ for i in bb.instructions]
        if pre is None and "InstRegisterMove" in names:
            pre = bb
        elif pre is not None and "InstDMACopy" in names:
            main = bb
            break
    if pre is None or main is None:
        return
    for eng_name in ("SP", "Activation"):
        eng = None
        ins_pt = None
        for j, i in enumerate(pre.instructions):
            if str(getattr(i, "engine", "")).endswith(eng_name):
                eng = i.engine
                if after_first_rm:
                    if type(i).__name__ == "InstRegisterMove" \
                            and ins_pt is None:
                        ins_pt = j + 1
                elif type(i).__name__ == "InstEventSemaphore":
                    ins_pt = j + 1
        if ins_pt is None:
            continue
        moved, rest, stopped = [], [], False
        for i in main.instructions:
            if (not stopped and len(moved) < max_moved
                    and type(i).__name__ == "InstDMACopy"
                    and i.engine == eng):
                si = getattr(i, "sync_info", None)
                if si is None or not si.on_wait:
                    moved.append(i)
                    continue
                stopped = True
            rest.append(i)
        if moved:
            main.instructions[:] = rest
            pre.instructions[ins_pt:ins_pt] = moved


# ---------------------------------------------------------------- launch 1

def build_stream():
    nc = bass.Bass()
    hout_d = nc.dram_tensor("hout", [8, 256], F32, kind="ExternalOutput")
    emb_d = nc.dram_tensor("embp", [128, 4 * CH], BF16, kind="ExternalInput")
    blk_d = [nc.dram_tensor(f"w1p{b}", [128, 128 * n], BF16,
                            kind="ExternalInput")
             for b, n in enumerate(BLOCKS)]
    with tile.TileContext(nc) as tc:
        with tc.tile_pool(name="pool", bufs=1) as pool, \
             tc.tile_pool(name="psum", bufs=1, space="PSUM") as psum:
            emb = pool.tile([128, 4 * CH], BF16)
            nc.scalar.dma_start(emb[:], emb_d[:])
            w1 = pool.tile([128, 128 * CH], BF16)
            c0 = 0
            for b, nch in enumerate(BLOCKS):
                nc.sync.dma_start(w1[:, 128 * c0:128 * (c0 + nch)],
                                  blk_d[b][:])
                c0 += nch
            ps = psum.tile([8, 256], F32)
            for m in range(NMM):
                nc.tensor.matmul(ps[:], emb[:, 8 * m:8 * m + 8],
                                 w1[:, 256 * m:256 * m + 256],
                                 start=(m == 0), stop=(m == NMM - 1))
            hsb = pool.tile([8, 256], F32)
            nc.vector.tensor_copy(hsb[:], ps[:])
            nc.scalar.dma_start(hout_d[:], hsb[:])
    _split_multi_waits(nc)
    _hoist_preamble_dmas(nc, max_moved=4, after_first_rm=True)
    return nc


# ---------------------------------------------------------------- launch 2
# slot space: slot = 128k + i (i partition, k in 0..3); chunk k serves
# stream k//2.  The host permutes positions -> slots so that every
# duplicate-token group sits on ONE partition (emitter in the lowest
# stream-legal column, other members in spare columns of the same row),
# which turns the group-max into a few masked whole-column ops.

CS_B2 = 0          # b2 column [128, 4]
CS_G = 4           # gate column [128, 4]
CS_GA = 8          # g*BIG - BIG column [128, 4]
CS_ID = 12         # fp32 identity [128, 128]
CS_MSKM = 140      # up to 4 dup mask columns (1 on dup-source rows)
CS_MSKA = 144      # matching mask*BIG - BIG columns
CS_ONE = 148       # fp32 ones row [1, 128] on partition 0
CS_W = 276


def build_tail(mkey):
    """mkey: sorted tuple of (src_col, dst_col) masked-max ops."""
    nc = bass.Bass()
    rank_d = nc.dram_tensor("ranko", [1, 1024], F32, kind="ExternalOutput")
    val_d = nc.dram_tensor("valo", [128, 4], F32, kind="ExternalOutput")
    h_d = nc.dram_tensor("hq", [128, 4], BF16, kind="ExternalInput")
    w2_d = nc.dram_tensor("w2q", [128, 1024], BF16, kind="ExternalInput")
    cs_d = nc.dram_tensor("cs", [128, CS_W], F32, kind="ExternalInput")
    on_d = nc.dram_tensor("onesb", [128, 1], BF16, kind="ExternalInput")

    with tile.TileContext(nc) as tc:
        with tc.tile_pool(name="pool", bufs=1) as pool, \
             tc.tile_pool(name="scr", bufs=2) as scr, \
             tc.tile_pool(name="psum", bufs=1, space="PSUM") as psum:
            w2 = pool.tile([128, 1024], BF16)
            nc.sync.dma_start(w2[:], w2_d[:])
            cs = pool.tile([128, CS_W], F32)
            nc.sync.dma_start(cs[:], cs_d[:])
            hsb = pool.tile([128, 4], BF16)
            nc.scalar.dma_start(hsb[:], h_d[:])
            onesb = pool.tile([128, 1], BF16)
            nc.scalar.dma_start(onesb[:], on_d[:])
            ident = cs[:, CS_ID:CS_ID + 128]
            ones_row = cs[0:1, CS_ONE:CS_ONE + 128]

            # ---- PE warm-up: junk matmuls with no dependencies keep the
            # PE busy from the end of the preamble so HAM un-throttles
            # before the fp32 transpose/broadcast work ----
            wj = pool.tile([128, 512], BF16)
            nc.vector.memset(wj[:], 0.0)
            wp = psum.tile([1, 512], F32)
            for _ in range(4):
                nc.tensor.matmul(wp[:], wj[:, 0:1], wj[:],
                                 start=True, stop=True,
                                 skip_group_check=True)

            # ---- z column tile: 16 accumulating K=128 bf16 matmuls ----
            zc = psum.tile([128, 4], F32)
            first = True
            for k in range(4):
                s = k // 2
                for t in range(2):           # w2 hi/lo
                    for hc in range(2):      # h hi/lo
                        nc.tensor.matmul(
                            zc[:, k:k + 1],
                            w2[:, 512 * t + 128 * k:512 * t + 128 * k + 128],
                            hsb[:, 2 * s + hc:2 * s + hc + 1],
                            start=first, stop=(k == 3 and t == 1 and hc == 1),
                            skip_group_check=True)
                        first = False

            # ---- more warm-up filling the PE gap during the DVE phase ----
            for _ in range(5):
                nc.tensor.matmul(wp[:], wj[:, 0:1], wj[:],
                                 start=True, stop=True,
                                 skip_group_check=True)

            # ---- c = z + b2; masked same-row group maxes; gate ----
            c_col = pool.tile([128, 4], F32)
            nc.vector.tensor_tensor(c_col[:], zc[:], cs[:, CS_B2:CS_B2 + 4],
                                    Alu.add)
            for mi, (src, dst) in enumerate(mkey):
                msk = scr.tile([128, 1], F32, tag="msk")
                nc.vector.tensor_tensor(msk[:], c_col[:, src:src + 1],
                                        cs[:, CS_MSKM + mi:CS_MSKM + mi + 1],
                                        Alu.mult)
                nc.vector.tensor_tensor(msk[:], msk[:],
                                        cs[:, CS_MSKA + mi:CS_MSKA + mi + 1],
                                        Alu.add)
                nc.vector.tensor_tensor(c_col[:, dst:dst + 1],
                                        c_col[:, dst:dst + 1], msk[:],
                                        Alu.max)
            vt = pool.tile([128, 4], F32)
            nc.vector.tensor_tensor(vt[:], c_col[:], cs[:, CS_G:CS_G + 4],
                                    Alu.mult)
            vcol = pool.tile([128, 4], F32)
            nc.vector.tensor_tensor(vcol[:], vt[:], cs[:, CS_GA:CS_GA + 4],
                                    Alu.add)
            nc.scalar.dma_start(val_d[:], vcol[:])

            # ---- val row (4 ident-matmul transposes, one group) ----
            vr_ps = psum.tile([1, 512], F32)
            for k in range(4):
                nc.tensor.matmul(vr_ps[0:1, 128 * k:128 * k + 128],
                                 vcol[:, k:k + 1], ident,
                                 start=(k == 0), stop=(k == 3),
                                 skip_group_check=True)
            # ---- copy row chunks out of psum and broadcast each chunk as
            # soon as it lands (copy || matmul pipeline) ----
            vrow = pool.tile([1, 512], F32)
            cpB = psum.tile([128, 512], F32)
            for k in range(4):
                nc.vector.tensor_copy(vrow[0:1, 128 * k:128 * k + 128],
                                      vr_ps[0:1, 128 * k:128 * k + 128])
                nc.tensor.matmul(cpB[:, 128 * k:128 * k + 128], ones_row,
                                 vrow[0:1, 128 * k:128 * k + 128],
                                 start=(k == 0), stop=(k == 3),
                                 skip_group_check=True)

            # ---- rank: chunks 0-1 DVE is_lt counts, 2-3 ACT sign sums ----
            rkA = psum.tile([1, 512], F32)
            rkB = psum.tile([1, 512], F32)
            for k in range(2):
                G = scr.tile([128, 512], BF16, tag="G")
                nc.vector.tensor_scalar(G[:], cpB[:], vcol[:, k:k + 1],
                                        None, Alu.is_lt)
                nc.tensor.matmul(rkA[:], onesb[:], G[:],
                                 start=(k == 0), stop=(k == 1),
                                 skip_group_check=True)
            for k in range(2, 4):
                S = scr.tile([128, 512], BF16, tag="S")
                nc.scalar.activation(S[:], cpB[:], Act.Sign,
                                     bias=vcol[:, k:k + 1], scale=-1.0)
                nc.tensor.matmul(rkB[:], onesb[:], S[:],
                                 start=(k == 2), stop=(k == 3),
                                 skip_group_check=True)
            rk = pool.tile([1, 1024], F32)
            nc.vector.tensor_copy(rk[0:1, 0:512], rkA[:])
            nc.sync.dma_start(rank_d[0:1, 0:512], rk[0:1, 0:512])
            nc.scalar.copy(rk[0:1, 512:1024], rkB[:])
            nc.scalar.dma_start(rank_d[0:1, 512:1024], rk[0:1, 512:1024])
    _split_multi_waits(nc)
    _hoist_preamble_dmas(nc)
    return nc


_cache = {}


def _get_stream():
    if "stream" not in _cache:
        _cache["stream"] = build_stream()
    return _cache["stream"]


def _get_tail(mkey):
    key = ("tail", mkey)
    if key not in _cache:
        _cache[key] = build_tail(mkey)
    return _cache[key]


# ---------------------------------------------------------------- host side

def _bf16_hilo(x):
    """x float32 -> (hi, lo) bf16 arrays with hi + lo ~= x (RNE twice)."""
    hi = x.astype(BF)
    lo = (x - hi.astype(np.float32)).astype(BF)
    return hi, lo


def _host_prep_stream(padded, mc, emb, w1):
    flat0 = emb[padded].reshape(-1)            # [262144] f32
    flat1 = emb[mc].reshape(-1)
    per_core = []
    for i in range(NCORES):
        sl = slice(KSH * i, KSH * (i + 1))
        e0h, e0l = _bf16_hilo(flat0[sl].reshape(CH, 128))
        e1h, e1l = _bf16_hilo(flat1[sl].reshape(CH, 128))
        # embp[p, 4c + a], a: (s0h, s1h, s0l, s1l)
        ep = np.stack([e0h, e1h, e0l, e1l], axis=2)      # [CH, 128, 4]
        ep = np.ascontiguousarray(ep.transpose(1, 0, 2)).reshape(128, 4 * CH)
        wh, wl = _bf16_hilo(w1[sl].reshape(CH, 128, 64))
        wcat = np.concatenate([wh, wl], axis=2)          # [CH, 128, 128]
        wp = np.ascontiguousarray(wcat.transpose(1, 0, 2)).reshape(
            128, 128 * CH)
        m = {"embp": ep}
        c0 = 0
        for b, nch in enumerate(BLOCKS):
            m[f"w1p{b}"] = np.ascontiguousarray(
                wp[:, 128 * c0:128 * (c0 + nch)])
            c0 += nch
        per_core.append(m)
    return per_core


def _fold_h(houts, b1):
    hp = np.zeros((4, 128), np.float64)
    for r in houts:
        r = r.astype(np.float64)
        hp += r[0:4, 0:128] + r[4:8, 128:256]
    # rows (s0h, s1h, s0l, s1l); cols j<64 wh, j>=64 wl
    h_pre = (hp[0:2, 0:64] + hp[0:2, 64:128]
             + hp[2:4, 0:64] + hp[2:4, 64:128]).astype(np.float32)
    return np.maximum(h_pre + b1[None, :], 0.0)          # [2, 64] relu'd


def _host_prep_tail(comb, h, w2, b2):
    """comb: [512] int tokens; h: [2,64] f32 post-relu.

    Builds the position -> slot permutation: slot = 128k + i; chunk k
    serves stream k//2 (stream-0 positions fill columns 0-1, stream-1
    columns 2-3).  Every duplicate-token group is placed on a single
    partition row: the emitter in the lowest stream-legal column, the
    other members in spare stream-legal columns of the same row, giving
    whole-column masked max ops."""
    valid = comb != 0
    groups = {}
    seen = {}
    for q in range(512):
        t = int(comb[q])
        if t == 0:
            continue
        if t in seen:
            groups.setdefault(seen[t], []).append(q)
        else:
            seen[t] = q
    glist = [[f] + v for f, v in sorted(groups.items())]

    scols = {0: [0, 1], 1: [2, 3]}                # stream -> legal columns
    slot_of = np.full(512, -1, np.int64)
    used = np.zeros((128, 4), bool)
    mops = {}                                     # (src,dst) -> mask rows
    for gi, grp in enumerate(glist):
        assert gi < 128, "too many duplicate groups"
        free = {0: list(scols[0]), 1: list(scols[1])}
        em = grp[0]
        kd = free[em // 256].pop(0)
        slot_of[em] = 128 * kd + gi
        used[gi, kd] = True
        for q in grp[1:]:
            s = q // 256
            assert free[s], "duplicate group too large for one row"
            ks = free[s].pop(0)
            slot_of[q] = 128 * ks + gi
            used[gi, ks] = True
            mops.setdefault((ks, kd), []).append(gi)
    # fill the remaining positions into remaining stream-legal slots
    fill = {0: [], 1: []}
    for k in range(4):
        for i in range(128):
            if not used[i, k]:
                fill[k // 2].append(128 * k + i)
    fp = {0: 0, 1: 0}
    for q in range(512):
        if slot_of[q] < 0:
            s = q // 256
            slot_of[q] = fill[s][fp[s]]
            fp[s] += 1
    pos_of = np.empty(512, np.int64)
    pos_of[slot_of] = np.arange(512)

    mkey = tuple(sorted(mops.keys()))
    assert len(mkey) <= 4, "too many distinct mask ops"
    # emitter gate in slot order
    first = np.zeros(512, bool)
    for grp in glist:
        first[grp[0]] = True
    single = valid.copy()
    for grp in glist:
        for q in grp:
            single[q] = False
    g_pos = (single | first).astype(np.float32)          # emitters, by pos
    g_slot = g_pos[pos_of]                               # by slot
    gc = g_slot.reshape(4, 128).T                        # [128, 4]

    cs = np.zeros((128, CS_W), np.float32)
    b2q = np.concatenate([b2, b2])[pos_of]
    cs[:, CS_B2:CS_B2 + 4] = b2q.reshape(4, 128).T
    cs[:, CS_G:CS_G + 4] = gc
    cs[:, CS_GA:CS_GA + 4] = gc * np.float32(BIG) - np.float32(BIG)
    cs[:, CS_ID:CS_ID + 128] = np.eye(128, dtype=np.float32)
    for mi, sk in enumerate(mkey):
        m = np.zeros(128, np.float32)
        m[mops[sk]] = 1.0
        cs[:, CS_MSKM + mi] = m
        cs[:, CS_MSKA + mi] = m * np.float32(BIG) - np.float32(BIG)
    cs[0, CS_ONE:CS_ONE + 128] = 1.0

    hh, hl = _bf16_hilo(h)                               # [2, 64] each
    hq = np.zeros((128, 4), BF)                          # K padded to 128
    for s in range(2):
        hq[0:64, 2 * s + 0] = hh[s]
        hq[0:64, 2 * s + 1] = hl[s]
    w2h, w2l = _bf16_hilo(w2)                            # [64, 256]
    wcols = pos_of % 256                                 # w2 column per slot
    w2q = np.zeros((128, 1024), BF)
    w2q[0:64, 0:512] = w2h[:, wcols]
    w2q[0:64, 512:1024] = w2l[:, wcols]
    onesb = np.ones((128, 1), BF)
    return mkey, pos_of, {"hq": hq, "w2q": w2q, "cs": cs, "onesb": onesb}


def kernel(input_tokens, memory_context, emb_table, w1, b1, w2, b2,
           _trace=False, _tmpdir=None):
    it = np.asarray(input_tokens).astype(np.int64)
    mc = np.asarray(memory_context).astype(np.int64)
    emb = np.asarray(emb_table, dtype=np.float32)
    w1 = np.asarray(w1, dtype=np.float32)
    b1 = np.asarray(b1, dtype=np.float32)
    w2 = np.asarray(w2, dtype=np.float32)
    b2 = np.asarray(b2, dtype=np.float32)
    padded = np.zeros(MSL, np.int64)
    padded[:it.shape[0]] = it
    comb = np.concatenate([padded, mc])

    per_core = _host_prep_stream(padded, mc, emb, w1)
    nc1 = _get_stream()
    res1 = run_bass_kernel_spmd(nc1, per_core, core_ids=list(range(NCORES)),
                                trace=_trace, tmpdir=_tmpdir)
    h = _fold_h([r["hout"] for r in res1.results], b1)

    mkey, pos_of, tail_in = _host_prep_tail(comb, h, w2, b2)
    nc2 = _get_tail(mkey)
    res2 = run_bass_kernel_spmd(nc2, [tail_in], core_ids=[0], trace=_trace)
    out = res2.results[0]
    # slot-ordered results: slot = 128k + i -> valo[i, k]
    val_s = np.asarray(out["valo"], np.float64).T.reshape(512)
    ro = np.asarray(out["ranko"], np.float64)[0]
    rkA, rkB = ro[0:512], ro[512:1024]
    eq = np.zeros(512)
    eq[256:512] = 1.0                        # self-match inside ACT chunks
    rank_s = np.rint(rkA + (256.0 - eq + rkB) / 2.0).astype(np.int64)

    tokens = np.zeros(256, np.int32)
    scores = np.full(256, NEG, np.float32)
    toks_s = comb[pos_of]                    # token of each slot
    emit = (val_s > -5e19) & (rank_s >= 0) & (rank_s < 256)
    r = rank_s[emit]
    tokens[r] = toks_s[emit].astype(np.int32)
    scores[r] = (1.0 / (1.0 + np.exp(-val_s[emit]))).astype(np.float32)
    kernel.last_result = (res1, res2)
    return tokens, scores
